# revision 1
# baseline (speedup 1.0000x reference)
"""CenterLoss kernel for Trainium2 (8 NeuronCores, data-parallel over batch).

loss = mean_i ||x_i - centers[labels_i]||^2   with x [16384,512], centers [4000,512].

Instead of the reference's full [B,C] distance matrix, each core:
  - streams its 2048-row x shard in groups of G row-blocks ([128, G*512]
    tiles, G row-blocks side by side),
  - gathers the matching G*128 center rows per group with a single SWDGE
    dma_gather (ucode-generated descriptors; output lands partition-major,
    exactly matching the x layout), spreading gathers over the SWDGE queues,
  - computes (x - c) in place on the Vector engine, Square-with-accumulate on
    the Scalar engine -> per-partition partial sums,
  - reduces to a [128,1] partial-sum vector that the host combines.

Built on bacc.Bacc so finalize() legalizes the 1-sync-wait-per-instruction
hardware constraint (generate_event_semaphores). A tiny DVE "probe" copy
absorbs the x-DMA wait so the subtract mostly waits on the gather alone.
"""

import numpy as np

try:
    import concourse.bass as bass
except ModuleNotFoundError:  # fallback if the repo isn't on sys.path
    import sys

    sys.path.insert(0, "/opt/trn_rl_repo")
    import concourse.bass as bass

import concourse.bacc as bacc
import concourse.mybir as mybir
import concourse.tile as tile
from concourse.bass_utils import run_bass_kernel_spmd

B, C, D = 16384, 4000, 512
N_CORES = 8
BS = B // N_CORES  # 2048 rows per core
P = 128
NT = BS // P  # 16 row-blocks per core
# row-blocks per group (one x DMA + one dma_gather each); smaller tail
# groups shrink the exposed compute latency after the last gather
GROUPS = [2, 2, 2, 2, 2, 2, 2, 1, 1]
NG = len(GROUPS)
assert sum(GROUPS) == NT

_nc_cache = {}


def set_config(g):
    """Uniform group size (benchmarking experiments)."""
    global GROUPS, NG
    GROUPS = [g] * (NT // g)
    NG = len(GROUPS)


def build_bass(reps=1, nq=4, dual_hwdge=False, frontload=False, single_packet=True):
    # reps>1 repeats the computation (benchmarking only); nq = SWDGE queues.
    # dual_hwdge alternates x loads between the SP and ACT HWDGE rings.
    # frontload issues every DMA of a rep before the compute ops.
    nc = bacc.Bacc(num_swdge_queues=nq, dynamic_dma_scratch_size=65536)
    x_d = nc.declare_dram_parameter("x", [BS, D], mybir.dt.float32, isOutput=False)
    # wrapped int16 labels: within each group's 8*g columns, element
    # (k % 16, col0 + k//16) = labels[row0*P + k]; replicated to 128 partitions
    lab_d = nc.declare_dram_parameter(
        "labels16", [P, NT * 8], mybir.dt.int16, isOutput=False
    )
    ctr_d = nc.declare_dram_parameter("centers", [C, D], mybir.dt.float32, isOutput=False)
    # per-group per-partition partial sums; the host does the final reduce
    out_d = nc.declare_dram_parameter("out", [P, NG], mybir.dt.float32, isOutput=True)

    with tile.TileContext(nc) as tc:
        with (
            tc.tile_pool(name="const", bufs=1) as const_pool,
            tc.tile_pool(name="xp", bufs=NG) as xpool,
            tc.tile_pool(name="cp", bufs=NG) as cpool,
            tc.tile_pool(name="sp", bufs=NG) as spool,
            tc.tile_pool(name="pr", bufs=NG) as prpool,
        ):
            row0 = [sum(GROUPS[:t]) * P for t in range(NG)]  # first row of group t
            col0 = [sum(GROUPS[:t]) * 8 for t in range(NG)]  # first idx col of group t

            lab = const_pool.tile([P, NT * 8], mybir.dt.int16)
            nc.sync.dma_start(out=lab[:], in_=lab_d[:])
            ss_all = const_pool.tile([P, NG], mybir.dt.float32)

            def issue_dmas(t, i):
                g = GROUPS[t]
                xt = xpool.tile([P, g * D], mybir.dt.float32, tag="xt")
                # row-block n of this group lands in columns [n*D, (n+1)*D)
                xin = x_d[row0[t] : row0[t] + g * P, :].rearrange(
                    "(n p) d -> p n d", p=P
                )
                eng = nc.scalar if (dual_hwdge and i % 2) else nc.sync
                eng.dma_start(out=xt[:].rearrange("p (n d) -> p n d", d=D), in_=xin)
                ct = cpool.tile([P, g * D], mybir.dt.float32, tag="ct")
                nc.gpsimd.dma_gather(
                    out_ap=ct[:].rearrange("p (n d) -> p n d", d=D),
                    in_ap=ctr_d[:],
                    idxs_ap=lab[:, col0[t] : col0[t] + g * 8],
                    num_idxs=g * P,
                    num_idxs_reg=g * P,
                    elem_size=D,
                    queue_num=i % nq,
                    single_packet=single_packet,
                )
                return xt, ct

            def issue_compute(t, xt, ct):
                g = GROUPS[t]
                # probe: absorbs the x-DMA wait on the DVE queue so the
                # subtract right after it only carries the gather wait
                pr = prpool.tile([P, 1], mybir.dt.float32)
                nc.vector.tensor_copy(out=pr[:], in_=xt[:, 0:1])
                nc.vector.tensor_sub(xt[:], xt[:], ct[:])  # xt <- x - c
                sq = spool.tile([P, g * D], mybir.dt.float32, tag="sq")
                nc.scalar.activation(
                    out=sq[:],
                    in_=xt[:],
                    func=mybir.ActivationFunctionType.Square,
                    accum_out=ss_all[:, t : t + 1],
                )

            if frontload:
                for r in range(reps):
                    pending = [
                        (t, *issue_dmas(t, r * NG + t)) for t in range(NG)
                    ]
                    for t, xt, ct in pending:
                        issue_compute(t, xt, ct)
            else:
                for i in range(NG * reps):
                    t = i % NG
                    xt, ct = issue_dmas(t, i)
                    issue_compute(t, xt, ct)

            nc.sync.dma_start(out=out_d[:], in_=ss_all[:])
    return nc


def wrap_labels(ls):
    """[BS] int -> [P, NT*8] int16: per group, (k%16, col0 + k//16) = seg[k]."""
    parts = []
    off = 0
    for g in GROUPS:
        seg = ls[off : off + g * P]
        parts.append(seg.reshape(g * 8, 16).T)  # [16, 8g]
        off += g * P
    w = np.hstack(parts)  # [16, NT*8]
    return np.ascontiguousarray(np.tile(w, (P // 16, 1)).astype(np.int16))


# "shard": sort within each shard — consecutive gather descriptors hit
# near-consecutive center rows (stride ~2 => DRAM page locality) while still
# spanning all of centers (spread over DRAM banks). "global" (sort the whole
# batch before sharding) concentrates each core on a ~1MB centers slice and
# measured ~25% SLOWER (bank/channel contention). The mean is invariant under
# permuting (x row, label) pairs together, so either is exact.
SORT_BY_LABEL = "shard"


def shard_inputs(x, labels, centers):
    x = np.ascontiguousarray(np.asarray(x), dtype=np.float32)
    labels = np.asarray(labels).astype(np.int64)
    centers = np.ascontiguousarray(np.asarray(centers), dtype=np.float32)
    if SORT_BY_LABEL == "global":
        order = np.argsort(labels, kind="stable")
        x = x[order]
        labels = labels[order]
    in_maps = []
    for c in range(N_CORES):
        xs = x[c * BS : (c + 1) * BS]
        ls = labels[c * BS : (c + 1) * BS]
        if SORT_BY_LABEL == "shard":
            order = np.argsort(ls, kind="stable")
            xs = xs[order]
            ls = ls[order]
        in_maps.append(
            {
                "x": np.ascontiguousarray(xs),
                "labels16": wrap_labels(ls),
                "centers": centers,
            }
        )
    return in_maps


def run(x, labels, centers, trace=False, **kwargs):
    if "nc" not in _nc_cache:
        nc = build_bass()
        if not nc.is_finalized():
            nc.finalize()
        _nc_cache["nc"] = nc
    nc = _nc_cache["nc"]
    in_maps = shard_inputs(x, labels, centers)
    res = run_bass_kernel_spmd(nc, in_maps, list(range(N_CORES)), trace=trace, **kwargs)
    total = sum(float(r["out"].astype(np.float64).sum()) for r in res.results)
    return np.float32(total / B), res


def kernel(x, labels, centers):
    out, _ = run(x, labels, centers)
    return out

